# revision 1
# baseline (speedup 1.0000x reference)
"""LocalAttention3D Trainium2 kernel — Gram-shared, jk-sharded variant.

Each core takes (batch, jk-slice of 1024).  Scores for all 4 heads derive
from one Gram matrix per tile:
  S_h^T[lm, jk] = wk_h*wq_h*G[lm, jk] + wk_h*bq_h*s[lm]
                  + (terms constant along lm, which cancel in the softmax)
where G = x^T x and s = sum_d x.  So one G matmul per lm-tile feeds all 4
heads' exps (activation with per-head scale AP and per-partition bias AP
= SCALE*wk_h*bq_h*s[lm] - 33).  PE per tile: 1 G + 4 MMZ + 4 SEL +
4 MMAV = 13 matmuls vs 16 for the per-head-core layout.  Head sum happens
in the PSUM out accumulator.  Everything else follows kernel.py (v8):
Z via blockones matmul, Zinv = exp(-Ln(Z)) on Scalar, selector-matmul
broadcast, cast on DVE (some Scalar), normalize mult alternating DVE/Pool,
phase A of chunk jc software-pipelined against phase C of chunk jc-1.
"""

import math
import sys

sys.path.insert(0, "/opt/trn_rl_repo")

import numpy as np
import ml_dtypes

import bass_rust
import concourse.bass as bass
import concourse.tile as tile
from concourse import mybir
from concourse.bass_utils import run_bass_kernel_spmd

BF16 = ml_dtypes.bfloat16

B, D, HW = 2, 96, 64 * 64
NH = 4
NCORES = 8
SLC = 1024            # jk columns per core (4 slices x 2 batches)
JKC = 512             # jk columns per chunk
NJC = SLC // JKC      # 2 chunks per core
NT = HW // 128        # 32 lm-tiles
SCALE = 1.0 / math.sqrt(32.0)


def _split_excess_waits(nc, max_waits=1):
    ctr = 0
    for f in nc.m.functions:
        for blk in f.blocks:
            insts = blk.instructions
            out = []
            changed = False
            for ins in insts:
                try:
                    si = ins.sync_info
                except Exception:
                    si = None
                if si is not None and len(si.on_wait) > max_waits:
                    waits = list(si.on_wait)
                    for w in waits[:-max_waits]:
                        ctr += 1
                        nop = mybir.InstNoOp(
                            name=f"wsplit-{ctr}-{ins.name}", ins=[], outs=[])
                        nop.engine = ins.engine
                        nop.sync_info = bass_rust.SyncInfo(
                            on_wait=[w], on_update=[])
                        nc.register_instruction(nop, overwrite=True)
                        out.append(nop)
                        changed = True
                    ins.sync_info = bass_rust.SyncInfo(
                        on_wait=waits[-max_waits:], on_update=list(si.on_update))
                out.append(ins)
            if changed:
                blk.instructions = out


def _build_program():
    f32 = mybir.dt.float32
    bf16 = mybir.dt.bfloat16

    nc = bass.Bass("TRN2", target_bir_lowering=False, debug=False,
                   num_devices=1)
    xb_d = nc.dram_tensor("xb", [D, HW], bf16, kind="ExternalInput").ap()
    xq_d = nc.dram_tensor("xq", [D, SLC], bf16, kind="ExternalInput").ap()
    xt_d = nc.dram_tensor("xt", [128, NT * D], bf16,
                          kind="ExternalInput").ap()
    sr_d = nc.dram_tensor("sr", [128, NT], f32, kind="ExternalInput").ap()
    sc_d = nc.dram_tensor("sc", [128, 18], f32, kind="ExternalInput").ap()
    bo_d = nc.dram_tensor("bo", [128, NT * 64], bf16,
                          kind="ExternalInput").ap()
    se_d = nc.dram_tensor("se", [64, NT * 128], bf16,
                          kind="ExternalInput").ap()
    out_d = nc.dram_tensor("out", [D, SLC], bf16,
                           kind="ExternalOutput").ap()

    with tile.TileContext(nc) as tc:
        with (
            tc.tile_pool(name="cn", bufs=1) as cn,
            tc.tile_pool(name="ew", bufs=128) as ew,
            tc.tile_pool(name="zn", bufs=4) as zn,
            tc.tile_pool(name="zj", bufs=6) as zjp,
            tc.tile_pool(name="zs", bufs=6) as zsp,
            tc.tile_pool(name="pt", bufs=5) as ptp,
            tc.tile_pool(name="ob", bufs=2) as obp,
            tc.tile_pool(name="ps_s", bufs=2, space="PSUM") as ps_s,
            tc.tile_pool(name="ps_z", bufs=2, space="PSUM") as ps_z,
            tc.tile_pool(name="ps_b", bufs=3, space="PSUM") as ps_b,
            tc.tile_pool(name="ps_av", bufs=1, space="PSUM") as ps_av,
        ):
            XB = cn.tile([D, HW], bf16, tag="XB")
            XQ = cn.tile([D, SLC], bf16, tag="XQ")
            XT = cn.tile([128, NT * D], bf16, tag="XT")
            SR = cn.tile([128, NT], f32, tag="SR")
            SC = cn.tile([128, 18], f32, tag="SC")
            BO = cn.tile([128, NT * 64], bf16, tag="BO")
            SE = cn.tile([64, NT * 128], bf16, tag="SE")
            for tl, dr in ((XB, xb_d), (XQ, xq_d), (XT, xt_d), (SR, sr_d),
                           (SC, sc_d), (BO, bo_d), (SE, se_d)):
                nc.sync.dma_start(tl[:], dr[:])

            mult, add = mybir.AluOpType.mult, mybir.AluOpType.add
            VTH = cn.tile([128, NH * NT * D], bf16, tag="VTH")
            BIH = cn.tile([128, NH * NT], f32, tag="BIH")
            for h in range(NH):
                nc.vector.tensor_scalar(
                    VTH[:, h * NT * D:(h + 1) * NT * D], XT[:],
                    SC[:, 8 + h:9 + h], SC[:, 12 + h:13 + h], mult, add)
                nc.vector.tensor_scalar(
                    BIH[:, h * NT:(h + 1) * NT], SR[:],
                    SC[:, 4 + h:5 + h], SC[:, 16:17], mult, add)

            chunk_state = {}

            def emit_A(jc, t):
                s = chunk_state[jc]
                g = ps_s.tile([128, JKC], f32, tag="st")
                nc.tensor.matmul(g[:], XB[:, t * 128:(t + 1) * 128],
                                 XQ[:, jc * JKC:(jc + 1) * JKC],
                                 start=True, stop=True)
                row = []
                for h in range(NH):
                    et = ew.tile([128, JKC], bf16, tag="et",
                                 name=f"et{jc}_{t}_{h}")
                    nc.scalar.activation(
                        et[:], g[:], mybir.ActivationFunctionType.Exp,
                        scale=SC[:, h:h + 1], bias=BIH[:, h * NT + t:
                                                       h * NT + t + 1])
                    row.append(et)
                s["e"].append(row)

            def emit_MMZ(jc, t):
                s = chunk_state[jc]
                for h in range(NH):
                    zf = s["zf"][h // 2]
                    nc.tensor.matmul(zf[64 * (h % 2):64 * (h % 2) + 64, :],
                                     BO[:, t * 64:(t + 1) * 64],
                                     s["e"][t][h][:],
                                     start=(t == 0), stop=(t == NT - 1))

            def emit_lnz(jc):
                s = chunk_state[jc]
                for pair in range(2):
                    zl = zn.tile([128, JKC], f32, tag="zl",
                                 name=f"zl{jc}_{pair}")
                    nc.scalar.activation(zl[:], s["zf"][pair][:],
                                         mybir.ActivationFunctionType.Ln)
                    for g in range(2):
                        zib = zjp.tile([64, JKC], bf16, tag="zib",
                                       name=f"zib{jc}_{pair}_{g}")
                        nc.scalar.activation(
                            zib[:], zl[64 * g:64 * g + 64, :],
                            mybir.ActivationFunctionType.Exp, scale=-1.0)
                        s["zib"].append(zib)

            def emit_SEL(jc, t):
                s = chunk_state[jc]
                row = []
                for h in range(NH):
                    zb = ps_b.tile([128, JKC], f32, tag="zb")
                    nc.tensor.matmul(zb[:], SE[:, t * 128:(t + 1) * 128],
                                     s["zib"][h][:], start=True, stop=True)
                    zbs = zsp.tile([128, JKC], bf16, tag="zbs",
                                   name=f"zbs{jc}_{t}_{h}")
                    if (4 * t + h) % 9 < 2:
                        nc.scalar.copy(zbs[:], zb[:])
                    else:
                        nc.vector.tensor_copy(zbs[:], zb[:])
                    row.append(zbs)
                s["zp"].append(row)

            def emit_MULT(jc, t):
                s = chunk_state[jc]
                row = []
                for h in range(NH):
                    pt = ptp.tile([128, JKC], bf16, tag="pt",
                                  name=f"pt{jc}_{t}_{h}")
                    if (4 * t + h) % 2 == 0:
                        nc.gpsimd.tensor_mul(pt[:], s["e"][t][h][:],
                                             s["zp"][t][h][:])
                    else:
                        nc.vector.tensor_mul(pt[:], s["e"][t][h][:],
                                             s["zp"][t][h][:])
                    row.append(pt)
                s["p"].append(row)

            def emit_MMAV(jc, av, t):
                s = chunk_state[jc]
                for h in range(NH):
                    nc.tensor.matmul(
                        av[:], VTH[:, (h * NT + t) * D:(h * NT + t + 1) * D],
                        s["p"][t][h][:],
                        start=(t == 0 and h == 0),
                        stop=(t == NT - 1 and h == NH - 1))

            for jc in range(NJC + 1):
                if jc < NJC:
                    zfa = ps_z.tile([128, JKC], f32, tag="zf",
                                    name=f"zf{jc}_a")
                    zfb = ps_z.tile([128, JKC], f32, tag="zf",
                                    name=f"zf{jc}_b")
                    chunk_state[jc] = {"e": [], "zf": [zfa, zfb], "zib": [],
                                       "zp": [], "p": []}
                cjc = jc - 1
                if cjc >= 0:
                    av = ps_av.tile([D, JKC], f32, tag="av")
                for t in range(NT + 7):
                    if jc < NJC and t < NT:
                        emit_A(jc, t)
                    if cjc >= 0:
                        if t < NT:
                            emit_SEL(cjc, t)
                        if 2 <= t < NT + 2:
                            emit_MULT(cjc, t - 2)
                        if 5 <= t < NT + 5:
                            emit_MMAV(cjc, av, t - 5)
                    if jc < NJC:
                        if 3 <= t < NT + 3:
                            emit_MMZ(jc, t - 3)
                        if t - 3 == NT - 1:
                            emit_lnz(jc)
                if cjc >= 0:
                    ob = obp.tile([D, JKC], bf16, tag="ob")
                    nc.scalar.copy(ob[:], av[:])
                    nc.sync.dma_start(out_d[:, cjc * JKC:(cjc + 1) * JKC],
                                      ob[:])
                    del chunk_state[cjc]

    _split_excess_waits(nc)
    return nc


_NC = None


def _get_program():
    global _NC
    if _NC is None:
        _NC = _build_program()
    return _NC


def _make_in_maps(x, wq, bq, wk, bk, wv, bv):
    x = np.asarray(x, dtype=np.float32)
    x2 = x.reshape(B, D, HW)
    wq, bq, wk, bk, wv, bv = [np.asarray(a, dtype=np.float32)
                              for a in (wq, bq, wk, bk, wv, bv)]

    bones = np.zeros((128, NT * 64), dtype=BF16)
    for t in range(NT):
        for g in range(2):
            bones[g * 64:(g + 1) * 64, t * 64 + 2 * t + g] = BF16(1.0)
    sel = np.zeros((64, NT * 128), dtype=BF16)
    for t in range(NT):
        sel[2 * t, t * 128:t * 128 + 64] = BF16(1.0)
        sel[2 * t + 1, t * 128 + 64:t * 128 + 128] = BF16(1.0)

    sc = np.zeros((128, 18), dtype=np.float32)
    for h in range(NH):
        sc[:, h] = SCALE * wk[h] * wq[h]
        sc[:, 4 + h] = SCALE * wk[h] * bq[h]
        sc[:, 8 + h] = wv[h]
        sc[:, 12 + h] = bv[h]
    sc[:, 16] = -33.0

    in_maps = []
    for c in range(NCORES):
        b, sl = divmod(c, NH)
        xb = x2[b]
        xt = np.ascontiguousarray(
            xb.reshape(D, NT, 128).transpose(2, 1, 0).reshape(128, NT * D))
        srt = np.ascontiguousarray(
            xb.sum(axis=0).reshape(NT, 128).T).astype(np.float32)
        in_maps.append({
            "xb": xb.astype(BF16),
            "xq": np.ascontiguousarray(
                xb[:, sl * SLC:(sl + 1) * SLC]).astype(BF16),
            "xt": xt.astype(BF16),
            "sr": srt,
            "sc": sc,
            "bo": bones,
            "se": sel,
        })
    return in_maps


def kernel(x, wq, bq, wk, bk, wv, bv):
    nc = _get_program()
    in_maps = _make_in_maps(x, wq, bq, wk, bk, wv, bv)
    res = run_bass_kernel_spmd(nc, in_maps, core_ids=list(range(NCORES)))
    out = np.zeros((B, 1, D, 64, 64), dtype=np.float32)
    for c in range(NCORES):
        b, sl = divmod(c, NH)
        out[b, 0].reshape(D, HW)[:, sl * SLC:(sl + 1) * SLC] = \
            res.results[c]["out"].astype(np.float32)
    return out

